# revision 4
# baseline (speedup 1.0000x reference)
"""MoE (B=4,S=1024,D=1024,E=8,F=4096,top2) — expert-parallel Trainium2 kernel.

Strategy:
  * Host: router (softmax + top-2 + renorm) in numpy f32; dispatch tokens to
    experts ("all-to-all" done host-side as the sharding step).
  * Device: 8 cores SPMD, core e owns expert e.  Each core computes
    y = (gelu(x_e @ w1_e + b1_e) @ w2_e) * combine_weight  for its gathered
    tokens (padded to capacity C).  Matmuls in bf16 with f32 PSUM accumulate.
  * Host: scatter-add per-expert outputs + combine@b2 bias term + LB loss.
"""

import numpy as np
import ml_dtypes

import concourse.bacc as bacc
import concourse.mybir as mybir
from concourse.tile import TileContext
from concourse.bass_utils import run_bass_kernel_spmd

BF16 = mybir.dt.bfloat16
F32 = mybir.dt.float32

B, S, D, E, F = 4, 1024, 1024, 8, 4096
TOP_K = 2
LOAD_BALANCING_WEIGHT = 0.01

KD = D // 128  # 8 contraction tiles for matmul1
KF = F // 128  # 32 f tiles
FG = 8         # f-tiles per w1 DMA group (1024 cols)
N_CORES = 8

_module_cache: dict[int, object] = {}
_last_in_maps = None  # stashed for external profiling harnesses


def _nblocks(C):
    blocks = []
    off = 0
    while off < C:
        n = min(512, C - off)
        blocks.append((off, n))
        off += n
    return blocks


def _build_module(C):
    """Bass module for one expert over C tokens (C multiple of 128)."""
    nc = bacc.Bacc("TRN2")
    MT = C // 128

    xt = nc.declare_dram_parameter("xt", [D, C], BF16, isOutput=False)
    w1 = nc.declare_dram_parameter("w1", [D, F], BF16, isOutput=False)
    b1 = nc.declare_dram_parameter("b1", [128, KF], F32, isOutput=False)
    w2 = nc.declare_dram_parameter("w2", [F, D], BF16, isOutput=False)
    wt = nc.declare_dram_parameter("wt", [128, MT], F32, isOutput=False)
    y = nc.declare_dram_parameter("y", [C, D], F32, isOutput=True)

    nb = _nblocks(C)

    with TileContext(nc) as tc:
        with (
            tc.tile_pool(name="persist", bufs=1) as persist,
            tc.tile_pool(name="w1p", bufs=2) as w1p,
            tc.tile_pool(name="outp", bufs=4) as outp,
            tc.tile_pool(name="ps1", bufs=6, space="PSUM") as ps1,
            tc.tile_pool(name="ps2", bufs=2, space="PSUM") as ps2,
        ):
            xts = []
            for d in range(KD):
                xt_t = persist.tile([128, C], BF16, tag=f"xt{d}", name=f"xt{d}")
                nc.sync.dma_start(xt_t[:, :], xt[d * 128:(d + 1) * 128, :])
                xts.append(xt_t)
            b1_t = persist.tile([128, KF], F32, tag="b1", name="b1t")
            nc.sync.dma_start(b1_t[:, :], b1[:, :])
            wt_t = persist.tile([128, MT], F32, tag="wt", name="wtt")
            nc.sync.dma_start(wt_t[:, :], wt[:, :])
            w2ts = []
            for f in range(KF):
                w2_t = persist.tile([128, D], BF16, tag=f"w2_{f}", name=f"w2t{f}")
                nc.sync.dma_start(w2_t[:, :], w2[f * 128:(f + 1) * 128, :])
                w2ts.append(w2_t)
            hts = []
            for f in range(KF):
                ht = persist.tile([128, C], BF16, tag=f"ht{f}", name=f"ht{f}")
                hts.append(ht)

            # Phase 1: hT[f, :] = gelu(w1.T @ xT + b1)   (tokens on free dim)
            for fg in range(KF // FG):
                w1g = []
                for d in range(KD):
                    w1_t = w1p.tile([128, FG * 128], BF16, tag=f"w1_{d}",
                                    name=f"w1g{fg}d{d}")
                    nc.sync.dma_start(
                        w1_t[:, :],
                        w1[d * 128:(d + 1) * 128, fg * FG * 128:(fg + 1) * FG * 128])
                    w1g.append(w1_t)
                for fl in range(FG):
                    f = fg * FG + fl
                    for (n0, nlen) in nb:
                        pt = ps1.tile([128, nlen], F32, tag="ps1", name=f"p1_{f}_{n0}")
                        for d in range(KD):
                            nc.tensor.matmul(
                                pt[:, :],
                                w1g[d][:, fl * 128:(fl + 1) * 128],
                                xts[d][:, n0:n0 + nlen],
                                start=(d == 0), stop=(d == KD - 1))
                        nc.scalar.activation(
                            hts[f][:, n0:n0 + nlen], pt[:, :],
                            mybir.ActivationFunctionType.Gelu,
                            bias=b1_t[:, f:f + 1], scale=1.0)

            # Phase 2: y[m, :] = (hT[:, m].T @ w2) * wt[m]  (tokens on partitions)
            for m in range(MT):
                for dn in range(2):
                    pt2 = ps2.tile([128, 512], F32, tag="ps2", name=f"p2_{m}_{dn}")
                    for f in range(KF):
                        nc.tensor.matmul(
                            pt2[:, :],
                            hts[f][:, m * 128:(m + 1) * 128],
                            w2ts[f][:, dn * 512:(dn + 1) * 512],
                            start=(f == 0), stop=(f == KF - 1))
                    yo = outp.tile([128, 512], F32, tag="yo", name=f"yo_{m}_{dn}")
                    nc.vector.tensor_scalar_mul(yo[:, :], pt2[:, :], wt_t[:, m:m + 1])
                    nc.sync.dma_start(
                        y[m * 128:(m + 1) * 128, dn * 512:(dn + 1) * 512], yo[:, :])

    nc.compile()
    return nc


def _route(x_flat, router_w, router_b):
    """Replicates the reference router in f32 numpy (matches jax top_k)."""
    logits = x_flat @ router_w + router_b                      # [T, E] f32
    m = logits.max(-1, keepdims=True)
    p = np.exp(logits - m, dtype=np.float32)
    p = p / p.sum(-1, keepdims=True, dtype=np.float32)
    order = np.argsort(-p, axis=-1, kind="stable")             # ties -> lower idx
    top_i = order[:, :TOP_K]                                   # [T, K]
    top_v = np.take_along_axis(p, top_i, axis=-1)
    top_v = top_v / (top_v.sum(-1, keepdims=True) + np.float32(1e-8))
    return p, top_i, top_v


def kernel(x, router_w, router_b, w1, b1, w2, b2):
    x = np.asarray(x, dtype=np.float32)
    router_w = np.asarray(router_w, dtype=np.float32)
    router_b = np.asarray(router_b, dtype=np.float32)
    w1 = np.asarray(w1, dtype=np.float32)
    b1 = np.asarray(b1, dtype=np.float32)
    w2 = np.asarray(w2, dtype=np.float32)
    b2 = np.asarray(b2, dtype=np.float32)

    T = x.shape[0] * x.shape[1]
    x_flat = x.reshape(T, D)

    probs, top_i, top_v = _route(x_flat, router_w, router_b)

    idxs, wtss = [], []
    for e in range(E):
        sel = (top_i == e)                      # [T, K]
        idx = np.nonzero(sel.any(-1))[0]
        w_tok = (top_v * sel).sum(-1)           # combine weight for expert e
        idxs.append(idx)
        wtss.append(w_tok[idx].astype(np.float32))

    cap = max(128, max(len(i) for i in idxs))
    C = ((cap + 127) // 128) * 128
    MT = C // 128

    if C not in _module_cache:
        _module_cache[C] = _build_module(C)
    nc = _module_cache[C]

    in_maps = []
    for e in range(E):
        idx = idxs[e]
        cnt = len(idx)
        xt_full = np.zeros((D, C), dtype=ml_dtypes.bfloat16)
        if cnt:
            xt_full[:, :cnt] = x_flat[idx].T.astype(ml_dtypes.bfloat16)
        wt_full = np.zeros(C, dtype=np.float32)
        wt_full[:cnt] = wtss[e]
        in_maps.append({
            "xt": xt_full,
            "w1": w1[e].astype(ml_dtypes.bfloat16),
            "b1": np.ascontiguousarray(b1[e].reshape(KF, 128).T.astype(np.float32)),
            "w2": w2[e].astype(ml_dtypes.bfloat16),
            "wt": np.ascontiguousarray(wt_full.reshape(MT, 128).T),
        })

    global _last_in_maps
    _last_in_maps = in_maps

    res = run_bass_kernel_spmd(nc, in_maps, core_ids=list(range(N_CORES)))

    out_flat = np.zeros((T, D), dtype=np.float32)
    for e in range(E):
        idx = idxs[e]
        if len(idx):
            out_flat[idx] += res.results[e]["y"][:len(idx)]

    combine = np.zeros((T, E), dtype=np.float32)
    for e in range(E):
        combine[idxs[e], e] = wtss[e]
    out_flat += combine @ b2

    avg = probs.mean(axis=0, dtype=np.float32)
    lbl = np.float32(LOAD_BALANCING_WEIGHT) * np.var(avg, ddof=1).astype(np.float32)

    return out_flat.reshape(B, S, D), np.array(lbl, dtype=np.float32)


# revision 8
# speedup vs baseline: 1.0506x; 1.0506x over previous
"""MoE (B=4,S=1024,D=1024,E=8,F=4096,top2) — expert-parallel Trainium2 kernel.

Strategy:
  * Host: router (softmax + top-2 + renorm) in numpy f32; dispatch tokens to
    experts ("all-to-all" done host-side as the sharding step).
  * Device: 8 cores SPMD, core e owns expert e.  Each core computes
    y = (gelu(x_e @ w1_e + b1_e) @ w2_e) * combine_weight  for its gathered
    tokens (padded to capacity C).  Matmuls in bf16 with f32 PSUM accumulate.
  * Host: scatter-add per-expert outputs + combine@b2 bias term + LB loss.
"""

import numpy as np
import ml_dtypes

import concourse.bacc as bacc
import concourse.mybir as mybir
from concourse.tile import TileContext
from concourse.bass_utils import run_bass_kernel_spmd

BF16 = mybir.dt.bfloat16
F32 = mybir.dt.float32

B, S, D, E, F = 4, 1024, 1024, 8, 4096
TOP_K = 2
LOAD_BALANCING_WEIGHT = 0.01

KD = D // 128  # 8 contraction tiles for matmul1
KF = F // 128  # 32 f tiles
FG = 8         # f-tiles per w1 DMA group (1024 cols)
N_CORES = 8

_module_cache: dict[int, object] = {}
_last_in_maps = None  # stashed for external profiling harnesses


def _nblocks(C):
    blocks = []
    off = 0
    while off < C:
        n = min(512, C - off)
        blocks.append((off, n))
        off += n
    return blocks


def _build_module(C):
    """Bass module for one expert over C tokens (C multiple of 128)."""
    nc = bacc.Bacc("TRN2")
    MT = C // 128

    xt = nc.declare_dram_parameter("xt", [D, C], BF16, isOutput=False)
    w1 = nc.declare_dram_parameter("w1", [D, F], BF16, isOutput=False)
    b1 = nc.declare_dram_parameter("b1", [128, KF], F32, isOutput=False)
    w2 = nc.declare_dram_parameter("w2", [F, D], BF16, isOutput=False)
    wt = nc.declare_dram_parameter("wt", [128, MT], F32, isOutput=False)
    y = nc.declare_dram_parameter("y", [C, D], F32, isOutput=True)

    nb = _nblocks(C)

    with TileContext(nc) as tc:
        with (
            tc.tile_pool(name="persist", bufs=1) as persist,
            tc.tile_pool(name="w1p", bufs=2) as w1p,
            tc.tile_pool(name="outp", bufs=4) as outp,
            tc.tile_pool(name="ps1", bufs=4, space="PSUM") as ps1,
            tc.tile_pool(name="ps2", bufs=4, space="PSUM") as ps2,
        ):
            xts = []
            for d in range(KD):
                xt_t = persist.tile([128, C], BF16, tag=f"xt{d}", name=f"xt{d}")
                xts.append(xt_t)
            # chunked so the first n-block's matmuls start as soon as the
            # first 512 token columns land (subtile deps)
            for c0 in range(0, C, 512):
                cl = min(512, C - c0)
                for d in range(KD):
                    nc.sync.dma_start(xts[d][:, c0:c0 + cl],
                                      xt[d * 128:(d + 1) * 128, c0:c0 + cl])
            b1_t = persist.tile([128, KF], F32, tag="b1", name="b1t")
            nc.sync.dma_start(b1_t[:, :], b1[:, :])
            wt_t = persist.tile([128, MT], F32, tag="wt", name="wtt")
            nc.sync.dma_start(wt_t[:, :], wt[:, :])
            w2ts = []
            for f in range(KF):
                w2_t = persist.tile([128, D], BF16, tag=f"w2_{f}", name=f"w2t{f}")
                w2ts.append(w2_t)
            hts = []
            for f in range(KF):
                ht = persist.tile([128, C], BF16, tag=f"ht{f}", name=f"ht{f}")
                hts.append(ht)

            # Phase 1: hT[f, :] = gelu(w1.T @ xT + b1)   (tokens on free dim)
            # w2 resident loads are spread across the f-groups so they fill
            # DMA idle time instead of delaying the first w1 group.
            n_groups = KF // FG
            w2_per_group = (KF + n_groups - 2) // (n_groups - 1)
            for fg in range(n_groups):
                w1g = []
                for d in range(KD):
                    w1_t = w1p.tile([128, FG * 128], BF16, tag=f"w1_{d}",
                                    name=f"w1g{fg}d{d}")
                    for h0 in range(0, FG * 128, 512):
                        nc.sync.dma_start(
                            w1_t[:, h0:h0 + 512],
                            w1[d * 128:(d + 1) * 128,
                               fg * FG * 128 + h0:fg * FG * 128 + h0 + 512])
                    w1g.append(w1_t)
                if fg >= 1:
                    for f in range((fg - 1) * w2_per_group,
                                   min(KF, fg * w2_per_group)):
                        nc.sync.dma_start(w2ts[f][:, :], w2[f * 128:(f + 1) * 128, :])
                for fl in range(FG):
                    f = fg * FG + fl
                    for (n0, nlen) in nb:
                        pt = ps1.tile([128, nlen], F32, tag="ps1", name=f"p1_{f}_{n0}")
                        for d in range(KD):
                            nc.tensor.matmul(
                                pt[:, :],
                                w1g[d][:, fl * 128:(fl + 1) * 128],
                                xts[d][:, n0:n0 + nlen],
                                start=(d == 0), stop=(d == KD - 1))
                        nc.scalar.activation(
                            hts[f][:, n0:n0 + nlen], pt[:, :],
                            mybir.ActivationFunctionType.Gelu,
                            bias=b1_t[:, f:f + 1], scale=1.0)

            # Phase 2: y[m, :] = (hT[:, m].T @ w2) * wt[m]  (tokens on partitions)
            for m in range(MT):
                for dn in range(2):
                    pt2 = ps2.tile([128, 512], F32, tag="ps2", name=f"p2_{m}_{dn}")
                    for f in range(KF):
                        nc.tensor.matmul(
                            pt2[:, :],
                            hts[f][:, m * 128:(m + 1) * 128],
                            w2ts[f][:, dn * 512:(dn + 1) * 512],
                            start=(f == 0), stop=(f == KF - 1))
                    yo = outp.tile([128, 512], F32, tag="yo", name=f"yo_{m}_{dn}")
                    nc.vector.tensor_scalar_mul(yo[:, :], pt2[:, :], wt_t[:, m:m + 1])
                    nc.sync.dma_start(
                        y[m * 128:(m + 1) * 128, dn * 512:(dn + 1) * 512], yo[:, :])

    nc.compile()
    return nc


def _route(x_flat, router_w, router_b):
    """Replicates the reference router in f32 numpy (matches jax top_k)."""
    logits = x_flat @ router_w + router_b                      # [T, E] f32
    m = logits.max(-1, keepdims=True)
    p = np.exp(logits - m, dtype=np.float32)
    p = p / p.sum(-1, keepdims=True, dtype=np.float32)
    order = np.argsort(-p, axis=-1, kind="stable")             # ties -> lower idx
    top_i = order[:, :TOP_K]                                   # [T, K]
    top_v = np.take_along_axis(p, top_i, axis=-1)
    top_v = top_v / (top_v.sum(-1, keepdims=True) + np.float32(1e-8))
    return p, top_i, top_v


def kernel(x, router_w, router_b, w1, b1, w2, b2):
    x = np.asarray(x, dtype=np.float32)
    router_w = np.asarray(router_w, dtype=np.float32)
    router_b = np.asarray(router_b, dtype=np.float32)
    w1 = np.asarray(w1, dtype=np.float32)
    b1 = np.asarray(b1, dtype=np.float32)
    w2 = np.asarray(w2, dtype=np.float32)
    b2 = np.asarray(b2, dtype=np.float32)

    T = x.shape[0] * x.shape[1]
    x_flat = x.reshape(T, D)

    probs, top_i, top_v = _route(x_flat, router_w, router_b)

    idxs, wtss = [], []
    for e in range(E):
        sel = (top_i == e)                      # [T, K]
        idx = np.nonzero(sel.any(-1))[0]
        w_tok = (top_v * sel).sum(-1)           # combine weight for expert e
        idxs.append(idx)
        wtss.append(w_tok[idx].astype(np.float32))

    cap = max(128, max(len(i) for i in idxs))
    C = ((cap + 127) // 128) * 128
    MT = C // 128

    if C not in _module_cache:
        _module_cache[C] = _build_module(C)
    nc = _module_cache[C]

    in_maps = []
    for e in range(E):
        idx = idxs[e]
        cnt = len(idx)
        xt_full = np.zeros((D, C), dtype=ml_dtypes.bfloat16)
        if cnt:
            xt_full[:, :cnt] = x_flat[idx].T.astype(ml_dtypes.bfloat16)
        wt_full = np.zeros(C, dtype=np.float32)
        wt_full[:cnt] = wtss[e]
        in_maps.append({
            "xt": xt_full,
            "w1": w1[e].astype(ml_dtypes.bfloat16),
            "b1": np.ascontiguousarray(b1[e].reshape(KF, 128).T.astype(np.float32)),
            "w2": w2[e].astype(ml_dtypes.bfloat16),
            "wt": np.ascontiguousarray(wt_full.reshape(MT, 128).T),
        })

    global _last_in_maps
    _last_in_maps = in_maps

    res = run_bass_kernel_spmd(nc, in_maps, core_ids=list(range(N_CORES)))

    out_flat = np.zeros((T, D), dtype=np.float32)
    for e in range(E):
        idx = idxs[e]
        if len(idx):
            out_flat[idx] += res.results[e]["y"][:len(idx)]

    combine = np.zeros((T, E), dtype=np.float32)
    for e in range(E):
        combine[idxs[e], e] = wtss[e]
    out_flat += combine @ b2

    avg = probs.mean(axis=0, dtype=np.float32)
    lbl = np.float32(LOAD_BALANCING_WEIGHT) * np.var(avg, ddof=1).astype(np.float32)

    return out_flat.reshape(B, S, D), np.array(lbl, dtype=np.float32)


# revision 24
# speedup vs baseline: 1.0823x; 1.0302x over previous
"""MoE (B=4,S=1024,D=1024,E=8,F=4096,top2) — expert-parallel Trainium2 kernel.

Strategy:
  * Host: router (softmax + top-2 + renorm) in numpy f32; dispatch tokens to
    experts ("all-to-all" done host-side as the sharding step).
  * Device: 8 cores SPMD, core e owns expert e.  Each core computes
    y = (gelu(x_e @ w1_e + b1_e) @ w2_e) * combine_weight  for its gathered
    tokens (padded to capacity C).  Matmuls in bf16 with f32 PSUM accumulate.
  * Host: scatter-add per-expert outputs + combine@b2 bias term + LB loss.
"""

import numpy as np
import ml_dtypes

import concourse.bacc as bacc
import concourse.mybir as mybir
from concourse.tile import TileContext
from concourse.bass_utils import run_bass_kernel_spmd

BF16 = mybir.dt.bfloat16
F32 = mybir.dt.float32

B, S, D, E, F = 4, 1024, 1024, 8, 4096
TOP_K = 2
LOAD_BALANCING_WEIGHT = 0.01

KD = D // 128  # 8 contraction tiles for matmul1
KF = F // 128  # 32 f tiles
FG = 8         # f-tiles per w1 DMA group (1024 cols)
N_CORES = 8

_module_cache: dict[int, object] = {}
_last_in_maps = None  # stashed for external profiling harnesses


def _nblocks(C):
    blocks = []
    off = 0
    while off < C:
        n = min(512, C - off)
        blocks.append((off, n))
        off += n
    return blocks


def _build_module(C):
    """Bass module for one expert over C tokens (C multiple of 128)."""
    nc = bacc.Bacc("TRN2")
    MT = C // 128

    xt = nc.declare_dram_parameter("xt", [D, C], BF16, isOutput=False)
    # w1 is (f, d)-interleaved host-side: [128, KF*KD*128], so one 256KB DMA
    # delivers one f-tile's weights for ALL d (keeps first fetch lean).
    w1 = nc.declare_dram_parameter("w1", [128, KF * KD * 128], BF16,
                                   isOutput=False)
    b1 = nc.declare_dram_parameter("b1", [128, KF], F32, isOutput=False)
    w2 = nc.declare_dram_parameter("w2", [F, D], BF16, isOutput=False)
    wt = nc.declare_dram_parameter("wt", [128, MT], F32, isOutput=False)
    y = nc.declare_dram_parameter("y", [C, D], F32, isOutput=True)

    nb = _nblocks(C)

    with TileContext(nc) as tc:
        with (
            tc.tile_pool(name="persist", bufs=1) as persist,
            tc.tile_pool(name="w1p", bufs=6) as w1p,
            tc.tile_pool(name="outp", bufs=4) as outp,
            tc.tile_pool(name="ps1", bufs=4, space="PSUM") as ps1,
            tc.tile_pool(name="ps2", bufs=4, space="PSUM") as ps2,
        ):
            # PE warmup: ~64 junk matmuls on scratch tiles keep the PE busy
            # while the initial DMA fetch is in flight, so the HAM clock gate
            # is already at 2.4 GHz when the first real chain starts.
            wu_w = persist.tile([128, 128], BF16, tag="wu_w", name="wu_w")
            wu_x = persist.tile([128, 512], BF16, tag="wu_x", name="wu_x")
            nc.vector.memset(wu_w[:, :], 0.0)
            nc.vector.memset(wu_x[:, :], 0.0)
            wu_ps = ps2.tile([128, 512], F32, tag="ps2", name="wu_ps")
            for i in range(64):
                nc.tensor.matmul(wu_ps[:, :], wu_w[:, :], wu_x[:, :],
                                 start=(i == 0), stop=(i == 63))

            xts = []
            for d in range(KD):
                xt_t = persist.tile([128, C], BF16, tag=f"xt{d}", name=f"xt{d}")
                nc.sync.dma_start(xt_t[:, :], xt[d * 128:(d + 1) * 128, :])
                xts.append(xt_t)
            b1_t = persist.tile([128, KF], F32, tag="b1", name="b1t")
            nc.sync.dma_start(b1_t[:, :], b1[:, :])
            wt_t = persist.tile([128, MT], F32, tag="wt", name="wtt")
            nc.sync.dma_start(wt_t[:, :], wt[:, :])
            w2ts = []
            for f in range(KF):
                w2_t = persist.tile([128, D], BF16, tag=f"w2_{f}", name=f"w2t{f}")
                w2ts.append(w2_t)
            hts = []
            for f in range(KF):
                ht = persist.tile([128, C], BF16, tag=f"ht{f}", name=f"ht{f}")
                hts.append(ht)

            # Phase 1: hT[f, :] = gelu(w1.T @ xT + b1)   (tokens on free dim)
            # w1 streams one f-tile per DMA; w2 resident loads are trickled in
            # behind (4 per f-tile from f=4) so they never delay phase 1.
            for f in range(KF):
                w1f = w1p.tile([128, KD * 128], BF16, tag="w1f", name=f"w1f{f}")
                nc.sync.dma_start(w1f[:, :],
                                  w1[:, f * KD * 128:(f + 1) * KD * 128])
                for (n0, nlen) in nb:
                    pt = ps1.tile([128, nlen], F32, tag="ps1", name=f"p1_{f}_{n0}")
                    for d in range(KD):
                        nc.tensor.matmul(
                            pt[:, :],
                            w1f[:, d * 128:(d + 1) * 128],
                            xts[d][:, n0:n0 + nlen],
                            start=(d == 0), stop=(d == KD - 1))
                    nc.scalar.activation(
                        hts[f][:, n0:n0 + nlen], pt[:, :],
                        mybir.ActivationFunctionType.Gelu,
                        bias=b1_t[:, f:f + 1], scale=1.0)
                if f >= 4:
                    nc.sync.dma_start(w2ts[f - 4][:, :],
                                      w2[(f - 4) * 128:(f - 3) * 128, :])
            for f in range(KF - 4, KF):
                nc.sync.dma_start(w2ts[f][:, :], w2[f * 128:(f + 1) * 128, :])

            # Phase 2: y[m, :] = (hT[:, m].T @ w2) * wt[m]  (tokens on partitions)
            for m in range(MT):
                for dn in range(2):
                    pt2 = ps2.tile([128, 512], F32, tag="ps2", name=f"p2_{m}_{dn}")
                    for f in range(KF):
                        nc.tensor.matmul(
                            pt2[:, :],
                            hts[f][:, m * 128:(m + 1) * 128],
                            w2ts[f][:, dn * 512:(dn + 1) * 512],
                            start=(f == 0), stop=(f == KF - 1))
                    yo = outp.tile([128, 512], F32, tag="yo", name=f"yo_{m}_{dn}")
                    # ScalarE is idle in phase 2 and faster than DVE here
                    nc.scalar.mul(yo[:, :], pt2[:, :], wt_t[:, m:m + 1])
                    nc.sync.dma_start(
                        y[m * 128:(m + 1) * 128, dn * 512:(dn + 1) * 512], yo[:, :])

    nc.compile()
    return nc


def _route(x_flat, router_w, router_b):
    """Replicates the reference router in f32 numpy (matches jax top_k)."""
    logits = x_flat @ router_w + router_b                      # [T, E] f32
    m = logits.max(-1, keepdims=True)
    p = np.exp(logits - m, dtype=np.float32)
    p = p / p.sum(-1, keepdims=True, dtype=np.float32)
    order = np.argsort(-p, axis=-1, kind="stable")             # ties -> lower idx
    top_i = order[:, :TOP_K]                                   # [T, K]
    top_v = np.take_along_axis(p, top_i, axis=-1)
    top_v = top_v / (top_v.sum(-1, keepdims=True) + np.float32(1e-8))
    return p, top_i, top_v


def kernel(x, router_w, router_b, w1, b1, w2, b2):
    x = np.asarray(x, dtype=np.float32)
    router_w = np.asarray(router_w, dtype=np.float32)
    router_b = np.asarray(router_b, dtype=np.float32)
    w1 = np.asarray(w1, dtype=np.float32)
    b1 = np.asarray(b1, dtype=np.float32)
    w2 = np.asarray(w2, dtype=np.float32)
    b2 = np.asarray(b2, dtype=np.float32)

    T = x.shape[0] * x.shape[1]
    x_flat = x.reshape(T, D)

    probs, top_i, top_v = _route(x_flat, router_w, router_b)

    idxs, wtss = [], []
    for e in range(E):
        sel = (top_i == e)                      # [T, K]
        idx = np.nonzero(sel.any(-1))[0]
        w_tok = (top_v * sel).sum(-1)           # combine weight for expert e
        idxs.append(idx)
        wtss.append(w_tok[idx].astype(np.float32))

    cap = max(128, max(len(i) for i in idxs))
    C = ((cap + 127) // 128) * 128
    MT = C // 128

    if C not in _module_cache:
        _module_cache[C] = _build_module(C)
    nc = _module_cache[C]

    in_maps = []
    for e in range(E):
        idx = idxs[e]
        cnt = len(idx)
        xt_full = np.zeros((D, C), dtype=ml_dtypes.bfloat16)
        if cnt:
            xt_full[:, :cnt] = x_flat[idx].T.astype(ml_dtypes.bfloat16)
        wt_full = np.zeros(C, dtype=np.float32)
        wt_full[:cnt] = wtss[e]
        in_maps.append({
            "xt": xt_full,
            "w1": np.ascontiguousarray(
                w1[e].reshape(KD, 128, KF, 128).transpose(1, 2, 0, 3)
                .reshape(128, KF * KD * 128)).astype(ml_dtypes.bfloat16),
            "b1": np.ascontiguousarray(b1[e].reshape(KF, 128).T.astype(np.float32)),
            "w2": w2[e].astype(ml_dtypes.bfloat16),
            "wt": np.ascontiguousarray(wt_full.reshape(MT, 128).T),
        })

    global _last_in_maps
    _last_in_maps = in_maps

    res = run_bass_kernel_spmd(nc, in_maps, core_ids=list(range(N_CORES)))

    out_flat = np.zeros((T, D), dtype=np.float32)
    for e in range(E):
        idx = idxs[e]
        if len(idx):
            out_flat[idx] += res.results[e]["y"][:len(idx)]

    combine = np.zeros((T, E), dtype=np.float32)
    for e in range(E):
        combine[idxs[e], e] = wtss[e]
    out_flat += combine @ b2

    avg = probs.mean(axis=0, dtype=np.float32)
    lbl = np.float32(LOAD_BALANCING_WEIGHT) * np.var(avg, ddof=1).astype(np.float32)

    return out_flat.reshape(B, S, D), np.array(lbl, dtype=np.float32)


# revision 29
# speedup vs baseline: 1.1241x; 1.0386x over previous
"""MoE (B=4,S=1024,D=1024,E=8,F=4096,top2) — expert-parallel Trainium2 kernel.

Strategy:
  * Host: router (softmax + top-2 + renorm) in numpy f32; dispatch tokens to
    experts ("all-to-all" done host-side as the sharding step).
  * Device: 8 cores SPMD, core e owns expert e.  Each core computes
    y = (gelu(x_e @ w1_e + b1_e) @ w2_e) * combine_weight  for its gathered
    tokens (padded to capacity C).  Matmuls in bf16 with f32 PSUM accumulate.
  * Host: scatter-add per-expert outputs + combine@b2 bias term + LB loss.
"""

import numpy as np
import ml_dtypes

import concourse.bacc as bacc
import concourse.mybir as mybir
from concourse.tile import TileContext
from concourse.bass_utils import run_bass_kernel_spmd

BF16 = mybir.dt.bfloat16
F32 = mybir.dt.float32

B, S, D, E, F = 4, 1024, 1024, 8, 4096
TOP_K = 2
LOAD_BALANCING_WEIGHT = 0.01

KD = D // 128  # 8 contraction tiles for matmul1
KF = F // 128  # 32 f tiles
FG = 8         # f-tiles per w1 DMA group (1024 cols)
N_CORES = 8

_module_cache: dict[int, object] = {}
_last_in_maps = None  # stashed for external profiling harnesses


def _nblocks(C):
    blocks = []
    off = 0
    while off < C:
        n = min(512, C - off)
        blocks.append((off, n))
        off += n
    return blocks


def _build_module(C):
    """Bass module for one expert over C tokens (C multiple of 64)."""
    nc = bacc.Bacc("TRN2")
    MT = (C + 127) // 128

    xt = nc.declare_dram_parameter("xt", [D, C], BF16, isOutput=False)
    # w1 is (f, d)-interleaved host-side: [128, KF*KD*128], so one 256KB DMA
    # delivers one f-tile's weights for ALL d (keeps first fetch lean).
    w1 = nc.declare_dram_parameter("w1", [128, KF * KD * 128], BF16,
                                   isOutput=False)
    b1 = nc.declare_dram_parameter("b1", [128, KF], F32, isOutput=False)
    w2 = nc.declare_dram_parameter("w2", [F, D], BF16, isOutput=False)
    wt = nc.declare_dram_parameter("wt", [128, MT], F32, isOutput=False)
    y = nc.declare_dram_parameter("y", [C, D], F32, isOutput=True)

    nb = _nblocks(C)

    with TileContext(nc) as tc:
        with (
            tc.tile_pool(name="persist", bufs=1) as persist,
            tc.tile_pool(name="w1p", bufs=6) as w1p,
            tc.tile_pool(name="outp", bufs=4) as outp,
            tc.tile_pool(name="ps1", bufs=4, space="PSUM") as ps1,
            tc.tile_pool(name="ps2", bufs=4, space="PSUM") as ps2,
        ):
            xts = []
            for d in range(KD):
                xt_t = persist.tile([128, C], BF16, tag=f"xt{d}", name=f"xt{d}")
                nc.sync.dma_start(xt_t[:, :], xt[d * 128:(d + 1) * 128, :])
                xts.append(xt_t)
            b1_t = persist.tile([128, KF], F32, tag="b1", name="b1t")
            nc.sync.dma_start(b1_t[:, :], b1[:, :])
            wt_t = persist.tile([128, MT], F32, tag="wt", name="wtt")
            nc.sync.dma_start(wt_t[:, :], wt[:, :])
            w2ts = []
            for f in range(KF):
                w2_t = persist.tile([128, D], BF16, tag=f"w2_{f}", name=f"w2t{f}")
                w2ts.append(w2_t)
            hts = []
            for f in range(KF):
                ht = persist.tile([128, C], BF16, tag=f"ht{f}", name=f"ht{f}")
                hts.append(ht)

            # Phase 1: hT[f, :] = gelu(w1.T @ xT + b1)   (tokens on free dim)
            # w1 streams one f-tile per DMA; w2 resident loads are trickled in
            # behind (4 per f-tile from f=4) so they never delay phase 1.
            for f in range(KF):
                w1f = w1p.tile([128, KD * 128], BF16, tag="w1f", name=f"w1f{f}")
                nc.sync.dma_start(w1f[:, :],
                                  w1[:, f * KD * 128:(f + 1) * KD * 128])
                for (n0, nlen) in nb:
                    pt = ps1.tile([128, nlen], F32, tag="ps1", name=f"p1_{f}_{n0}")
                    for d in range(KD):
                        nc.tensor.matmul(
                            pt[:, :],
                            w1f[:, d * 128:(d + 1) * 128],
                            xts[d][:, n0:n0 + nlen],
                            start=(d == 0), stop=(d == KD - 1))
                    nc.scalar.activation(
                        hts[f][:, n0:n0 + nlen], pt[:, :],
                        mybir.ActivationFunctionType.Gelu,
                        bias=b1_t[:, f:f + 1], scale=1.0)
                if f >= 4:
                    nc.sync.dma_start(w2ts[f - 4][:, :],
                                      w2[(f - 4) * 128:(f - 3) * 128, :])
            for f in range(KF - 4, KF):
                nc.sync.dma_start(w2ts[f][:, :], w2[f * 128:(f + 1) * 128, :])

            # Phase 2: y[m, :] = (hT[:, m].T @ w2) * wt[m]  (tokens on partitions)
            # The last m-tile may be 64 rows — an M=64 matmul costs the same
            # N cycles as M=128, so a 64-granular capacity is free here.
            for m in range(MT):
                ms = min(128, C - m * 128)
                for dn in range(2):
                    pt2 = ps2.tile([ms, 512], F32, tag="ps2", name=f"p2_{m}_{dn}")
                    for f in range(KF):
                        nc.tensor.matmul(
                            pt2[:, :],
                            hts[f][:, m * 128:m * 128 + ms],
                            w2ts[f][:, dn * 512:(dn + 1) * 512],
                            start=(f == 0), stop=(f == KF - 1))
                    yo = outp.tile([128, 512], F32, tag="yo", name=f"yo_{m}_{dn}")
                    nc.vector.tensor_scalar_mul(yo[:ms, :], pt2[:, :],
                                                wt_t[:ms, m:m + 1])
                    nc.sync.dma_start(
                        y[m * 128:m * 128 + ms, dn * 512:(dn + 1) * 512],
                        yo[:ms, :])

    nc.compile()
    return nc


def _route(x_flat, router_w, router_b):
    """Replicates the reference router in f32 numpy (matches jax top_k)."""
    logits = x_flat @ router_w + router_b                      # [T, E] f32
    m = logits.max(-1, keepdims=True)
    p = np.exp(logits - m, dtype=np.float32)
    p = p / p.sum(-1, keepdims=True, dtype=np.float32)
    order = np.argsort(-p, axis=-1, kind="stable")             # ties -> lower idx
    top_i = order[:, :TOP_K]                                   # [T, K]
    top_v = np.take_along_axis(p, top_i, axis=-1)
    top_v = top_v / (top_v.sum(-1, keepdims=True) + np.float32(1e-8))
    return p, top_i, top_v


def kernel(x, router_w, router_b, w1, b1, w2, b2):
    x = np.asarray(x, dtype=np.float32)
    router_w = np.asarray(router_w, dtype=np.float32)
    router_b = np.asarray(router_b, dtype=np.float32)
    w1 = np.asarray(w1, dtype=np.float32)
    b1 = np.asarray(b1, dtype=np.float32)
    w2 = np.asarray(w2, dtype=np.float32)
    b2 = np.asarray(b2, dtype=np.float32)

    T = x.shape[0] * x.shape[1]
    x_flat = x.reshape(T, D)

    probs, top_i, top_v = _route(x_flat, router_w, router_b)

    idxs, wtss = [], []
    for e in range(E):
        sel = (top_i == e)                      # [T, K]
        idx = np.nonzero(sel.any(-1))[0]
        w_tok = (top_v * sel).sum(-1)           # combine weight for expert e
        idxs.append(idx)
        wtss.append(w_tok[idx].astype(np.float32))

    cap = max(128, max(len(i) for i in idxs))
    C = ((cap + 63) // 64) * 64
    MT = (C + 127) // 128

    if C not in _module_cache:
        _module_cache[C] = _build_module(C)
    nc = _module_cache[C]

    in_maps = []
    for e in range(E):
        idx = idxs[e]
        cnt = len(idx)
        xt_full = np.zeros((D, C), dtype=ml_dtypes.bfloat16)
        if cnt:
            xt_full[:, :cnt] = x_flat[idx].T.astype(ml_dtypes.bfloat16)
        wt_full = np.zeros(MT * 128, dtype=np.float32)
        wt_full[:cnt] = wtss[e]
        in_maps.append({
            "xt": xt_full,
            "w1": np.ascontiguousarray(
                w1[e].reshape(KD, 128, KF, 128).transpose(1, 2, 0, 3)
                .reshape(128, KF * KD * 128)).astype(ml_dtypes.bfloat16),
            "b1": np.ascontiguousarray(b1[e].reshape(KF, 128).T.astype(np.float32)),
            "w2": w2[e].astype(ml_dtypes.bfloat16),
            "wt": np.ascontiguousarray(wt_full.reshape(MT, 128).T),
        })

    global _last_in_maps
    _last_in_maps = in_maps

    res = run_bass_kernel_spmd(nc, in_maps, core_ids=list(range(N_CORES)))

    out_flat = np.zeros((T, D), dtype=np.float32)
    for e in range(E):
        idx = idxs[e]
        if len(idx):
            out_flat[idx] += res.results[e]["y"][:len(idx)]

    combine = np.zeros((T, E), dtype=np.float32)
    for e in range(E):
        combine[idxs[e], e] = wtss[e]
    out_flat += combine @ b2

    avg = probs.mean(axis=0, dtype=np.float32)
    lbl = np.float32(LOAD_BALANCING_WEIGHT) * np.var(avg, ddof=1).astype(np.float32)

    return out_flat.reshape(B, S, D), np.array(lbl, dtype=np.float32)
